# revision 1
# baseline (speedup 1.0000x reference)
"""ColumnParallelLinearWithLoRA Trainium2 kernel.

Problem: out = x @ W^T + bias + per-token-LoRA, with
  x (4096, 4096) f32, W (4096, 4096) f32, bias (4096,) f32,
  lora_a (16, 16, 4096), lora_b (16, 4096, 16), indices (4096,) in [-1, 16).

Strategy (8 cores): row-parallel on tokens T — each core owns T/8 = 512
tokens end-to-end (base matmul + its own LoRA shrink/expand), which gives a
perfect FLOP split with zero replicated compute (vs. the column-parallel
hint, which replicates the shrink on every core).  The per-token LoRA gather
is reformulated as dense matmuls:

  tmpT[lr, t] = sum_h A_r[lr, h] * x[t, h]          (shrink; A_r = A.reshape(L*R, H))
  tmT         = tmpT * onehotT[lr, t]               (mask; onehot of indices, 0 for -1)
  out[t, o]   = sum_h x[t,h] W[o,h] + bias[o] + sum_lr tmT[lr,t] * B_r[lr,o]

where B_r[l*R+r, o] = lora_b[l, o, r].  All matmuls keep the same
orientation (stationary [K, t-or-lr], moving [K, o]), so no on-chip
transposes; operands are pre-transposed/tiled on the host.  bias is folded
in as a K=1 matmul of ones^T @ bias_row into the same PSUM accumulation.

Compute dtype bf16 (fp32 PSUM accumulate): fp32 matmul is 4 cycles/row on
TRN2 while bf16 is 1; measured L2 rel-err ~3e-3.
"""

import sys

sys.path.insert(0, "/opt/trn_rl_repo")

import numpy as np
import ml_dtypes

import concourse.bass as bass
import concourse.tile as tile
from concourse import bacc, mybir
from concourse.bass_utils import run_bass_kernel_spmd

T, H, O, L, R = 4096, 4096, 4096, 16, 16
N_CORES = 8
TS = T // N_CORES          # 512 tokens per core
P = 128
H_CHUNKS = H // P          # 32
O_SLICES = O // 512        # 8
T_TILES = TS // P          # 4
LR = L * R                 # 256
LR_TILES = LR // P         # 2

BF16 = ml_dtypes.bfloat16
DT = mybir.dt.bfloat16
F32 = mybir.dt.float32


def build_program(repeats: int = 1):
    """Build + compile the per-core Bass program (same program on all cores)."""
    nc = bacc.Bacc("TRN2", debug=False, enable_asserts=False)

    xt = nc.dram_tensor("xt", [P, H_CHUNKS * TS], DT, kind="ExternalInput").ap()
    wt = nc.dram_tensor("wt", [H_CHUNKS, O_SLICES, P, 512], DT, kind="ExternalInput").ap()
    at = nc.dram_tensor("at", [P, H_CHUNKS * LR], DT, kind="ExternalInput").ap()
    bt = nc.dram_tensor("bt", [LR_TILES, P, O], DT, kind="ExternalInput").ap()
    mk = nc.dram_tensor("mk", [LR_TILES, P, TS], DT, kind="ExternalInput").ap()
    bs = nc.dram_tensor("bs", [1, O], DT, kind="ExternalInput").ap()
    out = nc.dram_tensor("out", [TS, O], F32, kind="ExternalOutput").ap()

    from contextlib import ExitStack

    with tile.TileContext(nc) as tc, ExitStack() as ctx:
        const = ctx.enter_context(tc.tile_pool(name="const", bufs=1))
        psum = ctx.enter_context(tc.tile_pool(name="psum", bufs=8, space="PSUM"))
        wpool = ctx.enter_context(tc.tile_pool(name="wpool", bufs=6))
        opool = ctx.enter_context(tc.tile_pool(name="opool", bufs=6))

        ones_sb = const.tile([1, P], DT)
        nc.vector.memset(ones_sb[:], 1.0)

        for _rep in range(repeats):
            # resident inputs
            x_sb = const.tile([P, H_CHUNKS * TS], DT, tag="x")
            nc.sync.dma_start(x_sb[:], xt[:])
            a_sb = const.tile([P, H_CHUNKS * LR], DT, tag="a")
            nc.sync.dma_start(a_sb[:], at[:])
            b_sb = []
            for lt in range(LR_TILES):
                b_t = const.tile([P, O], DT, tag=f"b{lt}")
                nc.sync.dma_start(b_t[:], bt[lt][:])
                b_sb.append(b_t)
            m_sb = []
            for lt in range(LR_TILES):
                m_t = const.tile([P, TS], DT, tag=f"m{lt}")
                nc.sync.dma_start(m_t[:], mk[lt][:])
                m_sb.append(m_t)
            bias_sb = const.tile([1, O], DT, tag="bias")
            nc.sync.dma_start(bias_sb[:], bs[:])

            tm_sb = const.tile([P, LR_TILES * TS], DT, tag="tm")

            # ---- shrink: tmT[lr, t] = sum_h A_r[lr, h] x[t, h], then mask ----
            for lt in range(LR_TILES):
                ps_s = psum.tile([P, TS], F32, tag="ps")
                for c in range(H_CHUNKS):
                    nc.tensor.matmul(
                        ps_s[:],
                        lhsT=a_sb[:, c * LR + lt * P : c * LR + lt * P + P],
                        rhs=x_sb[:, c * TS : (c + 1) * TS],
                        start=(c == 0),
                        stop=(c == H_CHUNKS - 1),
                    )
                nc.vector.tensor_mul(
                    tm_sb[:, lt * TS : (lt + 1) * TS], ps_s[:], m_sb[lt][:]
                )

            # ---- main: out[t, o] = bias + x@W^T + tmT^T @ B_r ----
            for o_i in range(O_SLICES):
                osl = slice(o_i * 512, (o_i + 1) * 512)
                pts = []
                for tt in range(T_TILES):
                    pt = psum.tile([P, 512], F32, tag="ps")
                    # bias broadcast: ones(1,128)^T @ bias_row(1,512)
                    nc.tensor.matmul(
                        pt[:], lhsT=ones_sb[:, :], rhs=bias_sb[:, osl],
                        start=True, stop=False,
                    )
                    pts.append(pt)
                for c in range(H_CHUNKS):
                    w_t = wpool.tile([P, 512], DT, tag="w")
                    nc.sync.dma_start(w_t[:], wt[c, o_i][:])
                    for tt in range(T_TILES):
                        nc.tensor.matmul(
                            pts[tt][:],
                            lhsT=x_sb[:, c * TS + tt * P : c * TS + (tt + 1) * P],
                            rhs=w_t[:],
                            start=False, stop=False,
                        )
                for lt in range(LR_TILES):
                    for tt in range(T_TILES):
                        nc.tensor.matmul(
                            pts[tt][:],
                            lhsT=tm_sb[:, lt * TS + tt * P : lt * TS + (tt + 1) * P],
                            rhs=b_sb[lt][:, osl],
                            start=False, stop=(lt == LR_TILES - 1),
                        )
                for tt in range(T_TILES):
                    o_t = opool.tile([P, 512], F32, tag="o")
                    nc.vector.tensor_copy(o_t[:], pts[tt][:])
                    nc.sync.dma_start(
                        out[tt * P : (tt + 1) * P, osl], o_t[:]
                    )

    nc.compile()
    return nc


def prep_inputs(x, weight, bias, lora_a_stacked, lora_b_stacked, indices):
    """Host-side shard + layout prep. Returns per-core input maps."""
    x = np.asarray(x, dtype=np.float32)
    weight = np.asarray(weight, dtype=np.float32)
    bias = np.asarray(bias, dtype=np.float32)
    lora_a = np.asarray(lora_a_stacked, dtype=np.float32)
    lora_b = np.asarray(lora_b_stacked, dtype=np.float32)
    indices = np.asarray(indices)

    # W^T tiled: (H, O) -> (H_CHUNKS, O_SLICES, 128, 512), contiguous per tile
    wtb = weight.T.astype(BF16)  # (H, O)
    w_pre = np.ascontiguousarray(
        wtb.reshape(H_CHUNKS, P, O_SLICES, 512).transpose(0, 2, 1, 3)
    )

    # A_r^T: (H, LR) -> [128, c*LR + lr] layout
    a_rt = lora_a.reshape(LR, H).T.astype(BF16)  # (H, LR)
    a_pre = np.ascontiguousarray(
        a_rt.reshape(H_CHUNKS, P, LR).transpose(1, 0, 2)
    ).reshape(P, H_CHUNKS * LR)

    # B_r: lora_b (L, O, R) -> B_r[l*R+r, o] -> (LR_TILES, 128, O)
    b_r = np.ascontiguousarray(lora_b.transpose(0, 2, 1)).reshape(LR, O).astype(BF16)
    b_pre = np.ascontiguousarray(b_r.reshape(LR_TILES, P, O))

    bias_pre = bias.astype(BF16).reshape(1, O)

    in_maps = []
    for c in range(N_CORES):
        xs = x[c * TS : (c + 1) * TS, :]  # (TS, H)
        xts = xs.T.astype(BF16)  # (H, TS)
        x_pre = np.ascontiguousarray(
            xts.reshape(H_CHUNKS, P, TS).transpose(1, 0, 2)
        ).reshape(P, H_CHUNKS * TS)

        idx_s = indices[c * TS : (c + 1) * TS]
        onehot = (idx_s[None, :] == np.arange(L)[:, None]).astype(BF16)  # (L, TS)
        mk_pre = np.ascontiguousarray(
            np.repeat(onehot, R, axis=0).reshape(LR_TILES, P, TS)
        )

        in_maps.append(
            {
                "xt": x_pre,
                "wt": w_pre,
                "at": a_pre,
                "bt": b_pre,
                "mk": mk_pre,
                "bs": bias_pre,
            }
        )
    return in_maps


_PROGRAM_CACHE = {}


def kernel(x, weight, bias, lora_a_stacked, lora_b_stacked, indices):
    if "nc" not in _PROGRAM_CACHE:
        _PROGRAM_CACHE["nc"] = build_program()
    nc = _PROGRAM_CACHE["nc"]
    in_maps = prep_inputs(x, weight, bias, lora_a_stacked, lora_b_stacked, indices)
    res = run_bass_kernel_spmd(nc, in_maps, list(range(N_CORES)))
    return np.concatenate([res.results[c]["out"] for c in range(N_CORES)], axis=0)
